# revision 1
# baseline (speedup 1.0000x reference)
"""LiteMLA block, data-parallel over batch across 8 NeuronCores.

Sharding: B=8 batch elements -> one per core (all convs, linear attention,
and the 9x9 kv contraction are batch-independent); small weights and
pos_enc are replicated to every core. Inputs are FULL tensors; output is
the FULL (8,256,56,56) tensor gathered from the 8 shards.
"""
import numpy as np
import jax
import jax.numpy as jnp
from jax import lax

EPS = 1e-15
DIM = 8
HEADS = 32
HEADQ = 2 * HEADS
BN_EPS = 1e-5

B, C, H, W = 8, 256, 56, 56
N_CORES = 8


def _conv2d(x, w, groups=1, pad=0):
    return lax.conv_general_dilated(
        x, w, (1, 1), [(pad, pad), (pad, pad)],
        feature_group_count=groups,
        dimension_numbers=('NCHW', 'OIHW', 'NCHW'))


def _l2n(x):
    return x / (jnp.linalg.norm(x, axis=-1, keepdims=True) + EPS)


def _forward(x, w_qkv, w_dw, w_pw, pos_enc, ones_scale1,
             bn_gamma, bn_beta, bn_mean, bn_var,
             w_proj, pbn_gamma, pbn_beta, pbn_mean, pbn_var):
    # x: (b_local, C, H, W) for this core's batch shard
    b, _, h, w = x.shape
    n = h * w
    qkv = _conv2d(x, w_qkv)
    tmp = _conv2d(qkv, w_dw, groups=768, pad=2)
    tmp = _conv2d(tmp, w_pw, groups=96)
    ms = jnp.concatenate([qkv, tmp], axis=1)
    t = ms.reshape(b, HEADQ, 3 * DIM, n).transpose(0, 1, 3, 2)
    q, k, v = t[..., :DIM], t[..., DIM:2 * DIM], t[..., 2 * DIM:]
    pos = pos_enc.reshape(1, HEADQ, DIM, n).transpose(0, 1, 3, 2)
    k = k + pos
    q = _l2n(_l2n(q) ** 2)
    k = _l2n(_l2n(k) ** 2)
    ones = ones_scale1 * jnp.ones((b, HEADQ, n, 1), q.dtype)
    q = jnp.concatenate([q, ones], axis=-1)
    k = jnp.concatenate([k, ones], axis=-1)
    v1 = jnp.concatenate([v, jnp.ones((b, HEADQ, n, 1), v.dtype)], axis=-1)
    kv = jnp.einsum('bhnc,bhnd->bhcd', k, v1)
    out = jnp.einsum('bhnc,bhcd->bhnd', q, kv)
    out = out[..., :-1] / (out[..., -1:] + EPS)
    fm = v1[..., :-1].reshape(b * HEADQ, h, w, DIM)
    fm = (fm - bn_mean) * (bn_gamma / jnp.sqrt(bn_var + BN_EPS)) + bn_beta
    fm = jax.nn.gelu(fm, approximate=False).reshape(b, HEADQ, n, DIM)
    out = out + fm
    out = out.transpose(0, 1, 3, 2).reshape(b, HEADQ * DIM, h, w)
    out = _conv2d(out, w_proj)
    out = (out - pbn_mean[:, None, None]) * (
        pbn_gamma[:, None, None] / jnp.sqrt(pbn_var[:, None, None] + BN_EPS)
    ) + pbn_beta[:, None, None]
    return out


_pmapped = None


def _get_pmapped():
    global _pmapped
    if _pmapped is None:
        # batch axis sharded across cores; every other operand replicated
        _pmapped = jax.pmap(
            _forward,
            axis_name='b',
            in_axes=(0,) + (None,) * 14,
            devices=jax.devices()[:N_CORES],
        )
    return _pmapped


def kernel(**inputs: np.ndarray) -> np.ndarray:
    x = np.asarray(inputs['x'], np.float32)
    assert x.shape == (B, C, H, W)
    # shard: (8, C, H, W) -> (8 cores, 1, C, H, W)
    x_sh = x.reshape(N_CORES, 1, C, H, W)
    args = (
        x_sh,
        np.asarray(inputs['w_qkv'], np.float32),
        np.asarray(inputs['w_dw'], np.float32),
        np.asarray(inputs['w_pw'], np.float32),
        np.asarray(inputs['pos_enc'], np.float32),
        np.asarray(inputs['ones_scale1'], np.float32),
        np.asarray(inputs['bn_gamma'], np.float32),
        np.asarray(inputs['bn_beta'], np.float32),
        np.asarray(inputs['bn_mean'], np.float32),
        np.asarray(inputs['bn_var'], np.float32),
        np.asarray(inputs['w_proj'], np.float32),
        np.asarray(inputs['pbn_gamma'], np.float32),
        np.asarray(inputs['pbn_beta'], np.float32),
        np.asarray(inputs['pbn_mean'], np.float32),
        np.asarray(inputs['pbn_var'], np.float32),
    )
    try:
        out = _get_pmapped()(*args)        # (8, 1, 256, 56, 56)
        out = np.asarray(jax.device_get(out), np.float32)
        return out.reshape(B, 256, H, W)
    except Exception:
        return _forward_np(*((x,) + args[1:]))


def _forward_np(x, w_qkv, w_dw, w_pw, pos_enc, ones_scale1,
                bn_gamma, bn_beta, bn_mean, bn_var,
                w_proj, pbn_gamma, pbn_beta, pbn_mean, pbn_var):
    """Pure-numpy fallback (exact same math)."""
    from scipy.special import erf  # noqa: F401 — only if available
    b, c, h, w = x.shape
    n = h * w
    xf = x.reshape(b, c, n)
    qkv = np.einsum('oc,bcn->bon', w_qkv[:, :, 0, 0], xf)          # (b,768,n)
    # depthwise 5x5, pad 2
    qi = qkv.reshape(b, 768, h, w)
    qp = np.zeros((b, 768, h + 4, w + 4), np.float32)
    qp[:, :, 2:-2, 2:-2] = qi
    tmp = np.zeros_like(qi)
    for dy in range(5):
        for dx in range(5):
            tmp += w_dw[None, :, 0, dy, dx, None, None] * qp[:, :, dy:dy + h, dx:dx + w]
    # grouped 1x1, 96 groups of 8
    tg = tmp.reshape(b, 96, 8, n)
    wg = w_pw[:, :, 0, 0].reshape(96, 8, 8)
    tmp2 = np.einsum('goi,bgin->bgon', wg, tg).reshape(b, 768, n)
    ms = np.concatenate([qkv, tmp2], axis=1)
    t = ms.reshape(b, HEADQ, 3 * DIM, n).transpose(0, 1, 3, 2)
    q, k, v = t[..., :DIM], t[..., DIM:2 * DIM], t[..., 2 * DIM:]
    pos = pos_enc.reshape(1, HEADQ, DIM, n).transpose(0, 1, 3, 2)
    k = k + pos

    def l2n(z):
        return z / (np.linalg.norm(z, axis=-1, keepdims=True) + EPS)

    q = l2n(l2n(q) ** 2)
    k = l2n(l2n(k) ** 2)
    ones = np.float32(ones_scale1) * np.ones((b, HEADQ, n, 1), np.float32)
    q9 = np.concatenate([q, ones], axis=-1)
    k9 = np.concatenate([k, ones], axis=-1)
    v9 = np.concatenate([v, np.ones((b, HEADQ, n, 1), np.float32)], axis=-1)
    kv = np.einsum('bhnc,bhnd->bhcd', k9, v9)
    out = np.einsum('bhnc,bhcd->bhnd', q9, kv)
    out = out[..., :-1] / (out[..., -1:] + EPS)
    fm = v9[..., :-1]
    sc = bn_gamma / np.sqrt(bn_var + BN_EPS)
    fm = (fm - bn_mean) * sc + bn_beta
    from math import sqrt
    fm = fm * 0.5 * (1.0 + _erf_np(fm / np.float32(sqrt(2.0))))
    out = out + fm
    out = out.transpose(0, 1, 3, 2).reshape(b, HEADQ * DIM, n)
    out = np.einsum('oc,bcn->bon', w_proj[:, :, 0, 0], out)
    psc = pbn_gamma / np.sqrt(pbn_var + BN_EPS)
    out = (out - pbn_mean[None, :, None]) * psc[None, :, None] + pbn_beta[None, :, None]
    return out.reshape(b, 256, h, w).astype(np.float32)


def _erf_np(z):
    try:
        from scipy.special import erf
        return erf(z).astype(np.float32)
    except Exception:
        import math
        vec = np.vectorize(math.erf, otypes=[np.float32])
        return vec(z)

